# revision 4
# baseline (speedup 1.0000x reference)
"""Trainium2 Bass kernel for FASTMultiHeadAttention (degree-2 Taylor softmax
approximation with relative position bias).

  s_ij  = 1 + t + 0.5 t^2 = 0.5 (t+1)^2 + 0.5,   t_ij = q_i.k_j + q_i.rpe[i-j+N-1]
  o_i   = sum_j s_ij v_j / sum_j s_ij
        = (sum_j (t+1)^2 v_j + colsum(V)) / (sum_j (t+1)^2 + N)

Sharding: batch*head (16 heads) split over 8 cores, 2 heads per core.
The final (num + colsumV) / (den + N) divide runs on the host.

Per-core dataflow, software-pipelined across the 16 (head, i-block) tiles.
For block k (stages lag so every engine's per-iteration work only depends
on results produced in earlier iterations):
  iter k   : PE P'[p,t] = q_i . rpe_f[c0+t] (1152-col window, 3 matmuls);
             DVE+ACT copy P' PSUM->SBUF bf16; DMA write to per-block DRAM
             scratch (sync seq); sheared DMA read back G[p,j] = P'[p,127+j-p]
             (gpsimd seq)
  iter k+3 : PE qk1 = [Q;1]^T [K;1] (ones row folds +1); DVE t1 = qk1 + G
  iter k+4 : PE transpose t1 into the dead qk PSUM tile (bitcast bf16);
             ACT st = t1^2
  iter k+5 : PE O[i, 0:65] = sum_c st_c^T [V_c | 1]; ACT bounce PSUM->SBUF;
             DMA out (gpsimd seq)
"""

import numpy as np
import ml_dtypes
from contextlib import ExitStack

import concourse.bass as bass
import concourse.mybir as mybir
import concourse.tile as tile
from concourse import bacc, bass_utils
from concourse.masks import make_identity

B, H, N, D = 2, 8, 1024, 64
BH = B * H
NCORES = 8
HPC = BH // NCORES  # heads per core
NB = N // 128       # i-blocks per head
NBLK = HPC * NB     # total i-blocks per core
W = 1152            # P' window width (1151 needed, padded)
BF = mybir.dt.bfloat16
F32 = mybir.dt.float32
BF_NP = ml_dtypes.bfloat16

TRACE = False
_cached_nc = None

CP = 512   # P' copy split: DVE does [0:CP], ACT does [CP:W]
L1, L15, L2 = 3, 4, 5  # stage lags


def _build():
    nc = bacc.Bacc("TRN2", target_bir_lowering=False, debug=False,
                   num_devices=NCORES)
    # qa/ka: [65, HPC*N] = q^T (and k^T) per head with a ones row appended
    qa = nc.dram_tensor("qa", [65, HPC * N], BF, kind="ExternalInput").ap()
    ka = nc.dram_tensor("ka", [65, HPC * N], BF, kind="ExternalInput").ap()
    # rpet: flipped rpe^T, padded to 2048 cols
    rpet = nc.dram_tensor("rpet", [D, 2048], BF, kind="ExternalInput").ap()
    # vaug: [128, HPC*NB*65] -- V chunks with ones column appended
    vaug_d = nc.dram_tensor("vaug", [128, HPC * NB * 65], BF,
                            kind="ExternalInput").ap()
    # raw numerator|denominator per block; host finishes the divide
    o = nc.dram_tensor("o", [NBLK, 128, 65], F32, kind="ExternalOutput").ap()
    # per-block scratch tensors (separate so round trips don't false-serialize)
    scrs = [nc.dram_tensor(f"scr{k}", [128 * W], BF, kind="Internal")
            for k in range(NBLK)]

    with tile.TileContext(nc) as tc, ExitStack() as ctx:
        const = ctx.enter_context(tc.tile_pool(name="const", bufs=1))
        ppool = ctx.enter_context(tc.tile_pool(name="ppool", bufs=3))
        gpool = ctx.enter_context(tc.tile_pool(name="gpool", bufs=6))
        tpool = ctx.enter_context(tc.tile_pool(name="tpool", bufs=3))
        spool = ctx.enter_context(tc.tile_pool(name="spool", bufs=3))
        outp = ctx.enter_context(tc.tile_pool(name="outp", bufs=3))
        # PSUM banks: pp 3 + pqk 2x2 + po 1 = 8
        pp = ctx.enter_context(tc.tile_pool(name="pp", bufs=1, space="PSUM"))
        pqk = ctx.enter_context(tc.tile_pool(name="pqk", bufs=2, space="PSUM"))
        po = ctx.enter_context(tc.tile_pool(name="po", bufs=1, space="PSUM"))

        ident = const.tile([128, 128], BF, tag="ident")
        make_identity(nc, ident[:])

        qa_sb = const.tile([65, HPC * N], BF, tag="qa")
        nc.sync.dma_start(qa_sb[:], qa)
        rp_sb = const.tile([D, 2048], BF, tag="rp")
        nc.sync.dma_start(rp_sb[:], rpet)
        ka_sb = const.tile([65, HPC * N], BF, tag="ka")
        nc.sync.dma_start(ka_sb[:], ka)
        vaug = const.tile([128, HPC, NB, 65], BF, tag="vaug")
        nc.sync.dma_start(vaug[:], vaug_d)

        gs, t1s, sts = {}, {}, {}

        for k in range(NBLK + L2):
            if k < NBLK:
                # ---- stage A: P' window matmul, copy out, scratch round trip
                h, bi = divmod(k, NB)
                i0 = 128 * bi
                c0 = 896 - i0
                qblk = qa_sb[0:64, h * N + i0:h * N + i0 + 128]
                pps = pp.tile([128, W], F32, tag="pp")
                for off, wid in ((0, 512), (512, 512), (1024, 128)):
                    nc.tensor.matmul(pps[:, off:off + wid], qblk,
                                     rp_sb[:, c0 + off:c0 + off + wid],
                                     start=True, stop=True)
                p_sb = ppool.tile([128, W], BF, tag="p")
                nc.vector.tensor_copy(p_sb[:, 0:CP], pps[:, 0:CP])
                nc.scalar.activation(p_sb[:, CP:], pps[:, CP:],
                                     mybir.ActivationFunctionType.Copy)
                nc.sync.dma_start(bass.AP(scrs[k], 0, [[W, 128], [1, W]]),
                                  p_sb[:])
                g_sb = gpool.tile([128, N], BF, tag="g")
                nc.gpsimd.dma_start(
                    g_sb[:], bass.AP(scrs[k], 127, [[W - 1, 128], [1, N]]))
                gs[k] = g_sb

            if k >= L2:
                # ---- stage B2: PV matmuls, bounce to SBUF, store
                j = k - L2
                h, bi = divmod(j, NB)
                st = sts.pop(j)
                opsum = po.tile([128, 65], F32, tag="po")
                for c in range(8):
                    nc.tensor.matmul(opsum[:], st[:, 128 * c:128 * (c + 1)],
                                     vaug[:, h, c, :], start=(c == 0),
                                     stop=(c == 7))
                o_sb = outp.tile([128, 65], F32, tag="osb")
                nc.scalar.activation(o_sb[:], opsum[:],
                                     mybir.ActivationFunctionType.Copy)
                nc.gpsimd.dma_start(o[j], o_sb[:])

            if L1 <= k < NBLK + L1:
                # ---- stage B1: qk matmuls (+1 via ones row), add G
                j = k - L1
                h, bi = divmod(j, NB)
                i0 = 128 * bi
                qblk1 = qa_sb[:, h * N + i0:h * N + i0 + 128]
                qkt = pqk.tile([128, 1024], F32, tag="pqk")
                for jc in range(2):
                    nc.tensor.matmul(qkt[:, 512 * jc:512 * (jc + 1)], qblk1,
                                     ka_sb[:, h * N + 512 * jc:
                                           h * N + 512 * (jc + 1)],
                                     start=True, stop=True)
                t1 = tpool.tile([128, N], BF, tag="t1")
                nc.vector.tensor_add(t1[:], qkt[:], gs.pop(j)[:])
                t1s[j] = (t1, qkt)

            if L15 <= k < NBLK + L15:
                # ---- stage B15: transpose into dead qk banks, square
                j = k - L15
                t1, qkt = t1s.pop(j)
                tt = qkt[:, 0:512].bitcast(BF)  # [128, 1024] bf16 view
                for c in range(8):
                    nc.tensor.transpose(tt[:, 128 * c:128 * (c + 1)],
                                        t1[:, 128 * c:128 * (c + 1)],
                                        ident[:])
                st = spool.tile([128, N], BF, tag="st")
                nc.scalar.activation(st[:], tt[:],
                                     mybir.ActivationFunctionType.Square)
                sts[j] = st

    nc.compile()
    return nc


def kernel(**inputs):
    global _cached_nc
    q = np.asarray(inputs["q"], dtype=np.float32)
    k = np.asarray(inputs["k"], dtype=np.float32)
    v = np.asarray(inputs["v"], dtype=np.float32)
    rpe = np.asarray(inputs["rpe_matrix"], dtype=np.float32)

    qf = q.reshape(BH, N, D)
    kf = k.reshape(BH, N, D)
    vf = v.reshape(BH, N, D)

    # [BH, 65, N]: transposed q/k with a ones row appended
    qa = np.ones((BH, 65, N), dtype=BF_NP)
    qa[:, :64, :] = qf.transpose(0, 2, 1).astype(BF_NP)
    ka = np.ones((BH, 65, N), dtype=BF_NP)
    ka[:, :64, :] = kf.transpose(0, 2, 1).astype(BF_NP)

    rpet = np.zeros((D, 2048), dtype=BF_NP)
    rpet[:, :2047] = np.ascontiguousarray(rpe[::-1].T).astype(BF_NP)

    # vaug: [BH, 128, NB, 65] with ones col
    vaug = np.ones((BH, 128, NB, 65), dtype=BF_NP)
    vaug[:, :, :, :64] = vf.reshape(BH, NB, 128, D).transpose(0, 2, 1, 3
                                                              ).astype(BF_NP)

    if _cached_nc is None:
        _cached_nc = _build()
    nc = _cached_nc

    in_maps = []
    for c in range(NCORES):
        hs = slice(c * HPC, (c + 1) * HPC)
        in_maps.append({
            "qa": qa[hs].transpose(1, 0, 2).reshape(65, HPC * N),
            "ka": ka[hs].transpose(1, 0, 2).reshape(65, HPC * N),
            "rpet": rpet,
            "vaug": vaug[hs].transpose(1, 0, 2, 3).reshape(128,
                                                           HPC * NB * 65),
        })

    res = bass_utils.run_bass_kernel_spmd(
        nc, in_maps, core_ids=list(range(NCORES)), trace=TRACE)
    if TRACE:
        print(f"HW exec time: {res.exec_time_ns} ns")
        if res.instructions_and_trace is not None:
            print("trace:", res.instructions_and_trace[1])

    # assemble: raw[core, blk, p, 0:65] -> num/den divide on host
    raw = np.stack([r["o"] for r in res.results], axis=0)  # [8, NBLK, 128, 65]
    raw = raw.reshape(BH, NB * 128, 65)
    colsum = vf.sum(axis=1)  # [BH, 64]
    num = raw[:, :, :64] + colsum[:, None, :]
    den = raw[:, :, 64] + float(N)
    o = num / den[:, :, None]
    return o.reshape(B, H, N, D).astype(np.float32)


# revision 5
# speedup vs baseline: 1.2880x; 1.2880x over previous
"""Trainium2 Bass kernel for FASTMultiHeadAttention (degree-2 Taylor softmax
approximation with relative position bias).

  s_ij  = 1 + t + 0.5 t^2 = 0.5 (t+1)^2 + 0.5,   t_ij = q_i.k_j + q_i.rpe[i-j+N-1]
  o_i   = sum_j s_ij v_j / sum_j s_ij
        = (sum_j (t+1)^2 v_j + colsum(V)) / (sum_j (t+1)^2 + N)

Sharding: batch*head (16 heads) split over 8 cores, 2 heads per core.
The final (num + colsumV) / (den + N) divide runs on the host.

Per-core dataflow, software-pipelined across the 16 (head, i-block) tiles.
Stages lag so each engine's per-iteration work only depends on results from
earlier iterations:
  iter k   : PE P'[p,t] = q_i . rpe_f[c0+t] (1152-col window: 2x512 into a
             big-pool PSUM tile + 128 tail into its own bank); DVE+ACT copy
             P' to SBUF bf16; scratch write (sync seq); sheared read back
             G[p,j] = P'[p,127+j-p] (gpsimd seq)
  iter k+3 : PE qk1 = [Q;1]^T [K;1] (ones row folds the +1); DVE t1 = qk1+G
  iter k+4 : PE transpose t1 into the dead qk PSUM tile (bitcast bf16);
             ACT st = t1^2
  iter k+5 : PE O[i, 0:65] = sum_c st_c^T [V_c | 1]; DVE bounce PSUM->SBUF;
             DMA out (gpsimd seq)

PSUM banks: big pool 3x[128,1024]f32 (6) + P'-tail (1) + PV out (1) = 8.
"""

import numpy as np
import ml_dtypes
from contextlib import ExitStack

import concourse.bass as bass
import concourse.mybir as mybir
import concourse.tile as tile
from concourse import bacc, bass_utils
from concourse.masks import make_identity

B, H, N, D = 2, 8, 1024, 64
BH = B * H
NCORES = 8
HPC = BH // NCORES  # heads per core
NB = N // 128       # i-blocks per head
NBLK = HPC * NB     # total i-blocks per core
W = 1152            # P' window width (1151 needed, padded)
BF = mybir.dt.bfloat16
F32 = mybir.dt.float32
BF_NP = ml_dtypes.bfloat16

TRACE = False
_cached_nc = None

CP = 512   # P' copy split: DVE does [0:CP], ACT does [CP:1024] + tail
L1, L15, L2 = 3, 4, 5  # stage lags


def _build():
    nc = bacc.Bacc("TRN2", target_bir_lowering=False, debug=False,
                   num_devices=NCORES)
    # qa/ka: [65, HPC*N] = q^T (and k^T) per head with a ones row appended
    qa = nc.dram_tensor("qa", [65, HPC * N], BF, kind="ExternalInput").ap()
    ka = nc.dram_tensor("ka", [65, HPC * N], BF, kind="ExternalInput").ap()
    # rpet: flipped rpe^T, padded to 2048 cols
    rpet = nc.dram_tensor("rpet", [D, 2048], BF, kind="ExternalInput").ap()
    # vaug: [128, HPC*NB*65] -- V chunks with ones column appended
    vaug_d = nc.dram_tensor("vaug", [128, HPC * NB * 65], BF,
                            kind="ExternalInput").ap()
    # raw numerator|denominator per block; host finishes the divide
    o = nc.dram_tensor("o", [NBLK, 128, 65], F32, kind="ExternalOutput").ap()
    # per-block scratch tensors (separate so round trips don't false-serialize)
    scrs = [nc.dram_tensor(f"scr{k}", [128 * W], BF, kind="Internal")
            for k in range(NBLK)]

    with tile.TileContext(nc) as tc, ExitStack() as ctx:
        const = ctx.enter_context(tc.tile_pool(name="const", bufs=1))
        ppool = ctx.enter_context(tc.tile_pool(name="ppool", bufs=3))
        gpool = ctx.enter_context(tc.tile_pool(name="gpool", bufs=6))
        tpool = ctx.enter_context(tc.tile_pool(name="tpool", bufs=3))
        spool = ctx.enter_context(tc.tile_pool(name="spool", bufs=3))
        outp = ctx.enter_context(tc.tile_pool(name="outp", bufs=3))
        big = ctx.enter_context(tc.tile_pool(name="big", bufs=3, space="PSUM"))
        pm3 = ctx.enter_context(tc.tile_pool(name="pm3", bufs=1, space="PSUM"))
        po = ctx.enter_context(tc.tile_pool(name="po", bufs=1, space="PSUM"))

        ident = const.tile([128, 128], BF, tag="ident")
        make_identity(nc, ident[:])

        qa_sb = const.tile([65, HPC * N], BF, tag="qa")
        nc.sync.dma_start(qa_sb[:], qa)
        rp_sb = const.tile([D, 2048], BF, tag="rp")
        nc.sync.dma_start(rp_sb[:], rpet)
        ka_sb = const.tile([65, HPC * N], BF, tag="ka")
        nc.sync.dma_start(ka_sb[:], ka)
        vaug = const.tile([128, HPC, NB, 65], BF, tag="vaug")
        nc.sync.dma_start(vaug[:], vaug_d)

        gs, t1s, sts = {}, {}, {}

        for k in range(NBLK + L2):
            if k < NBLK:
                # ---- stage A: P' window matmuls, copy out, scratch round trip
                h, bi = divmod(k, NB)
                i0 = 128 * bi
                c0 = 896 - i0
                qblk = qa_sb[0:64, h * N + i0:h * N + i0 + 128]
                pa = big.tile([128, 1024], F32, tag="big")
                for jc in range(2):
                    nc.tensor.matmul(pa[:, 512 * jc:512 * (jc + 1)], qblk,
                                     rp_sb[:, c0 + 512 * jc:
                                           c0 + 512 * (jc + 1)],
                                     start=True, stop=True)
                pb = pm3.tile([128, 128], F32, tag="pm3")
                nc.tensor.matmul(pb[:], qblk, rp_sb[:, c0 + 1024:c0 + W],
                                 start=True, stop=True)
                p_sb = ppool.tile([128, W], BF, tag="p")
                nc.vector.tensor_copy(p_sb[:, 0:CP], pa[:, 0:CP])
                nc.scalar.activation(p_sb[:, CP:1024], pa[:, CP:1024],
                                     mybir.ActivationFunctionType.Copy)
                nc.scalar.activation(p_sb[:, 1024:W], pb[:],
                                     mybir.ActivationFunctionType.Copy)
                nc.sync.dma_start(bass.AP(scrs[k], 0, [[W, 128], [1, W]]),
                                  p_sb[:])
                g_sb = gpool.tile([128, N], BF, tag="g")
                nc.gpsimd.dma_start(
                    g_sb[:], bass.AP(scrs[k], 127, [[W - 1, 128], [1, N]]))
                gs[k] = g_sb

            if k >= L2:
                # ---- stage B2: PV matmuls, bounce to SBUF, store
                j = k - L2
                h, bi = divmod(j, NB)
                st = sts.pop(j)
                opsum = po.tile([128, 65], F32, tag="po")
                for c in range(8):
                    nc.tensor.matmul(opsum[:], st[:, 128 * c:128 * (c + 1)],
                                     vaug[:, h, c, :], start=(c == 0),
                                     stop=(c == 7))
                o_sb = outp.tile([128, 65], F32, tag="osb")
                nc.vector.tensor_copy(o_sb[:], opsum[:])
                nc.gpsimd.dma_start(o[j], o_sb[:])

            if L1 <= k < NBLK + L1:
                # ---- stage B1: qk matmuls (+1 via ones row), add G
                j = k - L1
                h, bi = divmod(j, NB)
                i0 = 128 * bi
                qblk1 = qa_sb[:, h * N + i0:h * N + i0 + 128]
                qkt = big.tile([128, 1024], F32, tag="big")
                for jc in range(2):
                    nc.tensor.matmul(qkt[:, 512 * jc:512 * (jc + 1)], qblk1,
                                     ka_sb[:, h * N + 512 * jc:
                                           h * N + 512 * (jc + 1)],
                                     start=True, stop=True)
                t1 = tpool.tile([128, N], BF, tag="t1")
                nc.vector.tensor_add(t1[:], qkt[:], gs.pop(j)[:])
                t1s[j] = (t1, qkt)

            if L15 <= k < NBLK + L15:
                # ---- stage B15: transpose into dead qk banks, square
                j = k - L15
                t1, qkt = t1s.pop(j)
                tt = qkt[:, 0:512].bitcast(BF)  # [128, 1024] bf16 view
                for c in range(8):
                    nc.tensor.transpose(tt[:, 128 * c:128 * (c + 1)],
                                        t1[:, 128 * c:128 * (c + 1)],
                                        ident[:])
                st = spool.tile([128, N], BF, tag="st")
                nc.scalar.activation(st[:], tt[:],
                                     mybir.ActivationFunctionType.Square)
                sts[j] = st

    nc.compile()
    return nc


def kernel(**inputs):
    global _cached_nc
    q = np.asarray(inputs["q"], dtype=np.float32)
    k = np.asarray(inputs["k"], dtype=np.float32)
    v = np.asarray(inputs["v"], dtype=np.float32)
    rpe = np.asarray(inputs["rpe_matrix"], dtype=np.float32)

    qf = q.reshape(BH, N, D)
    kf = k.reshape(BH, N, D)
    vf = v.reshape(BH, N, D)

    # [BH, 65, N]: transposed q/k with a ones row appended
    qa = np.ones((BH, 65, N), dtype=BF_NP)
    qa[:, :64, :] = qf.transpose(0, 2, 1).astype(BF_NP)
    ka = np.ones((BH, 65, N), dtype=BF_NP)
    ka[:, :64, :] = kf.transpose(0, 2, 1).astype(BF_NP)

    rpet = np.zeros((D, 2048), dtype=BF_NP)
    rpet[:, :2047] = np.ascontiguousarray(rpe[::-1].T).astype(BF_NP)

    # vaug: [BH, 128, NB, 65] with ones col
    vaug = np.ones((BH, 128, NB, 65), dtype=BF_NP)
    vaug[:, :, :, :64] = vf.reshape(BH, NB, 128, D).transpose(0, 2, 1, 3
                                                              ).astype(BF_NP)

    if _cached_nc is None:
        _cached_nc = _build()
    nc = _cached_nc

    in_maps = []
    for c in range(NCORES):
        hs = slice(c * HPC, (c + 1) * HPC)
        in_maps.append({
            "qa": qa[hs].transpose(1, 0, 2).reshape(65, HPC * N),
            "ka": ka[hs].transpose(1, 0, 2).reshape(65, HPC * N),
            "rpet": rpet,
            "vaug": vaug[hs].transpose(1, 0, 2, 3).reshape(128,
                                                           HPC * NB * 65),
        })

    res = bass_utils.run_bass_kernel_spmd(
        nc, in_maps, core_ids=list(range(NCORES)), trace=TRACE)
    if TRACE:
        print(f"HW exec time: {res.exec_time_ns} ns")
        if res.instructions_and_trace is not None:
            print("trace:", res.instructions_and_trace[1])

    # assemble: raw[core, blk, p, 0:65] -> num/den divide on host
    raw = np.stack([r["o"] for r in res.results], axis=0)  # [8, NBLK, 128, 65]
    raw = raw.reshape(BH, NB * 128, 65)
    colsum = vf.sum(axis=1)  # [BH, 64]
    num = raw[:, :, :64] + colsum[:, None, :]
    den = raw[:, :, 64] + float(N)
    o = num / den[:, :, None]
    return o.reshape(B, H, N, D).astype(np.float32)
